# revision 28
# baseline (speedup 1.0000x reference)
"""EquivariantEdgeConv fused Bass kernel for one TRN2 chip (8 NeuronCores).

Strategy (low-rank radial weights + fused TP/scatter on the PE):
  - The per-edge tensor-product weights w(len) = silu(len*w1) @ w2 / sqrt(H)
    lie on a 1-D curve parameterized by len.  An SVD of that curve (host
    side) shows rank R=6 reproduces it to ~1e-4, so each edge only needs
    R radial coefficients c[e,r] = h(len) @ V  (V = w2 @ B_R / sqrt(H)).
  - Per edge build a 256-wide feature row
        F = [ xs(48) | xv(48) | xv.y1(16) | xs x y1 (144) ]
    and the rank-1 expansion P[e, (r,f)] = c_r * F_f  (R*256 = 1536 cols,
    bf16, formed with R tensor_scalar ops at 4x DVE mode).
  - The tensor-product contraction is FUSED INTO THE SCATTER: per 128-edge
    tile, 12 PE matmuls accumulate Q[(r,f), n] += P_chunk^T @ onehot(dst)
    into PSUM; per 128-node bucket one [1536->96] weight contraction
    (12 accumulating matmuls against the host-built W_big) yields the
    scattered messages, followed by the gated o3.Linear node stage.
  - Nodes are sharded: core c owns nodes [1024c, 1024c+1024) as 8 buckets
    of 128; edges are bucketed by destination (host side) and padded to a
    shared static tile count.  x (+ a bf16-pair encoding of pos[src]) is
    fetched with ONE dma_gather per bucket; pos[dst] / dstloc / gather
    indices are streamed as contiguous per-bucket blocks.

Self-contained: shapes hardcoded for N=8192, E=65536, irreps 48x0e+16x1o,
H=64.
"""

import sys

if "/opt/trn_rl_repo" not in sys.path:
    sys.path.insert(0, "/opt/trn_rl_repo")

import numpy as np

import concourse.bacc as bacc
import concourse.bass as bass
import concourse.mybir as mybir
import concourse.tile as tile
from concourse.bass_utils import run_bass_kernel_spmd

M0, M1, H = 48, 16, 64
N_NODES, N_EDGES, N_CORES = 8192, 65536, 8
NODES_PER_CORE = N_NODES // N_CORES          # 1024
BUCKETS = NODES_PER_CORE // 128              # 8 buckets of 128 nodes per core
R = 5                                        # radial basis rank
GATHER_MODE = __import__('os').environ.get('GATHER_MODE', 'indirect')  # gather|indirect
NF = 256                                     # per-edge feature width
NCHUNK = R * NF // 128                       # 12 P/W chunks of 128 rows
FP = mybir.dt.float32
BF = mybir.dt.bfloat16
I16 = mybir.dt.int16
I32 = mybir.dt.int32

CA = 1.0 / np.sqrt(M0 * 2.0)
CB = 1.0 / np.sqrt(3.0 * M1 * 2.0)
CC = 1.0 / np.sqrt(M0 * 2.0)
CD = 1.0 / np.sqrt(M1 * 2.0)
SQRT3 = float(np.sqrt(3.0))


def _silu64(x):
    return x / (1.0 + np.exp(-x))


NKNOT = 64


def _radial_basis(w1, w2, len_max):
    """Rank-R SVD basis of the radial weight curve w(len), evaluated on
    device through a ReLU spline: c_r(len) = relu(len - knots) @ A.

    (ReLU is used because sqrt/relu/square/copy share one ACT function
    table set on TRN2 -- no per-tile table reloads.)

    Returns knots [NKNOT], A [NKNOT, R], BR [4096, R], all float64.
    """
    grid = np.linspace(0.0, float(len_max) * 1.02 + 1e-6, 2048)
    hs = _silu64(grid[:, None] * w1.astype(np.float64)[0][None, :])   # [S,H]
    ws = hs @ w2.astype(np.float64) / np.sqrt(H)                      # [S,4096]
    _, _, vt = np.linalg.svd(ws, full_matrices=False)
    br = vt[:R].T                                                     # [4096,R]
    c_true = ws @ br                                                  # [S,R]
    knots = np.linspace(0.0, float(len_max) * 1.01, NKNOT)
    g = np.maximum(grid[:, None] - knots[None, :], 0.0)               # [S,NKNOT]
    a = np.linalg.solve(
        g.T @ g + 1e-7 * np.eye(NKNOT), g.T @ c_true
    )                                                                 # [NKNOT,R]
    return knots, a, br


def _build_wbig(br):
    """W_big [(r,f), q] mapping rank-1 features to the 96 message outputs.

    f layout: [xs(48) | xv(i,m)(48) | xvy(16) | xsY(m-major,144)]
    q layout: [ms o<48 | pad(16) | mv 64+3o+m | pad(16)]  (mv at partition
    base 64 so the epilogue ACT copy reads at a legal partition offset)
    Returns [R*256, 128] float64.
    """
    wb = np.zeros((R, NF, 128))
    # a/b/c/d carry the trailing R axis ([i, o, R])
    a = br[:2304].reshape(48, 48, R)
    b = br[2304:3072].reshape(16, 48, R)
    c = br[3072:3840].reshape(48, 16, R)
    d = br[3840:4096].reshape(16, 16, R)
    for r in range(R):
        # path A: f=i (xs), q=o
        wb[r, 0:48, 0:48] = CA * a[:, :, r]
        # path B: f=96+i (xvy), q=o  (sqrt3 from Y1)
        wb[r, 96:112, 0:48] = CB * SQRT3 * b[:, :, r]
        # path D: f=48+3i+m (xv), q=64+3o+m
        for m in range(3):
            wb[r, 48 + m:96:3, 64 + m:112:3] = CD * d[:, :, r]
            # path C: f=112+48m+i (xsY), q=64+3o+m  (sqrt3 from Y1)
            wb[r, 112 + 48 * m:160 + 48 * m, 64 + m:112:3] = CC * SQRT3 * c[:, :, r]
    return wb.reshape(R * NF, 128)


def _wns_block(wns):
    """[48,48] lhsT for the 1o o3.Linear on (o,m)-interleaved rows."""
    out = np.zeros((48, 48), np.float32)
    for i in range(16):
        for m in range(3):
            for o in range(16):
                out[i * 3 + m, o * 3 + m] = wns[i, o] / np.sqrt(M1)
    return out


def _prep_edges(edge_index, pos):
    """Bucket/pad edges by destination.

    Returns per-core arrays:
      idx16  [N_CORES, BUCKETS*128, T*8]  int16  (dma_gather wrapped+replicated)
      dl     [N_CORES, BUCKETS*128, T]    fp32   (local dst, 300 for padding)
      pdst   [N_CORES, BUCKETS*128, T*4]  fp32   (pos[dst], w-padded)
    and the shared tiles-per-bucket count T.
    """
    src = edge_index[0].astype(np.int64)
    dst = edge_index[1].astype(np.int64)
    gb = dst >> 7
    order = np.argsort(gb, kind="stable")
    src_s, dst_s = src[order], dst[order]
    counts = np.bincount(gb[order], minlength=64)
    cap = max(int(np.ceil(counts.max() / 128) * 128), 128)
    T = cap // 128
    starts = np.concatenate([[0], np.cumsum(counts)])

    pos = np.asarray(pos, np.float32)
    idx16 = np.zeros((N_CORES, BUCKETS * 128, T * 8), np.int16)
    srcidx = np.zeros((N_CORES, BUCKETS * 128, T), np.int32)
    dl = np.full((N_CORES, BUCKETS * 128, T), 300.0, np.float32)
    pdst = np.zeros((N_CORES, BUCKETS * 128, T * 4), np.float32)

    for g in range(64):
        ccore, b = g >> 3, g & 7
        s, e = starts[g], starts[g + 1]
        n = e - s
        sidx = np.zeros(cap, np.int64)
        sidx[:n] = src_s[s:e]
        dloc = np.full(cap, 300.0, np.float32)
        dloc[:n] = (dst_s[s:e] - (g << 7)).astype(np.float32)
        pd = np.zeros((cap, 3), np.float32)
        pd[:n] = pos[dst_s[s:e]]
        pd[n:] = pos[0]  # padding: same as pos[src=0] so vec==0, no NaNs
        # edge k -> partition k%128, tile k//128
        k = np.arange(cap)
        p, t = k % 128, k // 128
        r0 = 128 * b
        dl[ccore, r0 + p, t] = dloc
        srcidx[ccore, r0 + p, t] = sidx.astype(np.int32)
        pdst[ccore, r0 + p[:, None], 4 * t[:, None] + np.arange(3)[None, :]] = pd
        # gather idx wrap: idx k -> [k%16, k//16], replicated to 128 partitions
        wrapped = np.zeros((16, T * 8), np.int16)
        wrapped[k % 16, k // 16] = sidx.astype(np.int16)
        idx16[ccore, r0:r0 + 128, :] = np.tile(wrapped, (8, 1))
    return (idx16, srcidx), dl, pdst, T


def build_kernel(tiles_per_bucket: int, reps: int = 1) -> bass.Bass:
    T = tiles_per_bucket
    assert T <= 10, "radial PSUM layout sized for T<=10"
    OHW = R * 128                         # scaled-onehot width (768)
    RHS_PARTS = [(0, min(512, OHW))] + ([(512, OHW)] if OHW > 512 else [])
    nc = bacc.Bacc(None, target_bir_lowering=False, debug=False)
    d_xb = nc.declare_dram_parameter("xb", [N_NODES, 128], BF, isOutput=False)
    d_idx = nc.declare_dram_parameter("idx16", [BUCKETS * 128, T * 8], I16, isOutput=False)
    d_srcidx = nc.declare_dram_parameter("srcidx", [BUCKETS * 128, T], I32, isOutput=False)
    d_dl = nc.declare_dram_parameter("dl", [BUCKETS * 128, T], FP, isOutput=False)
    d_pd = nc.declare_dram_parameter("pdst", [BUCKETS * 128, T * 4], FP, isOutput=False)
    d_wbig = nc.declare_dram_parameter("wbig", [128, NCHUNK * 128], FP, isOutput=False)
    d_v = nc.declare_dram_parameter("vmat", [NKNOT, R], FP, isOutput=False)
    d_knots = nc.declare_dram_parameter("nknots", [NKNOT, 1], FP, isOutput=False)
    d_ws = nc.declare_dram_parameter("ws", [M0, M0], FP, isOutput=False)
    d_wg = nc.declare_dram_parameter("wg", [M0, M0], FP, isOutput=False)
    d_wns = nc.declare_dram_parameter("wns", [48, 48], FP, isOutput=False)
    d_ident = nc.declare_dram_parameter("ident", [128, 128], FP, isOutput=False)
    d_iota = nc.declare_dram_parameter("iota", [128, 128], BF, isOutput=False)
    d_out = nc.declare_dram_parameter("out", [NODES_PER_CORE, M0], FP, isOutput=True)

    AF = mybir.ActivationFunctionType
    OP = mybir.AluOpType

    with tile.TileContext(nc) as tc, tc.tile_pool(name="consts", bufs=1) as cp:
        wbig_sb = cp.tile([128, NCHUNK * 128], FP)
        v_sb = cp.tile([NKNOT, R], FP)
        knots_sb = cp.tile([NKNOT, 1], FP)
        ws_sb = cp.tile([M0, M0], FP)
        wg_sb = cp.tile([M0, M0], FP)
        wns_sb = cp.tile([48, 48], FP)
        ident_sb = cp.tile([128, 128], FP)
        iota_sb = cp.tile([128, 128], BF)
        for sb, dr in (
            (wbig_sb, d_wbig), (v_sb, d_v), (knots_sb, d_knots), (ws_sb, d_ws),
            (wg_sb, d_wg), (wns_sb, d_wns), (ident_sb, d_ident), (iota_sb, d_iota),
        ):
            nc.sync.dma_start(out=sb[:], in_=dr[:])

        with (
            tc.tile_pool(name="bkt", bufs=2) as bktp,
            tc.tile_pool(name="geo", bufs=2) as geop,
            tc.tile_pool(name="fall", bufs=2) as fallp,
            tc.tile_pool(name="ohp", bufs=3) as ohp,
            tc.tile_pool(name="epi", bufs=2) as epip,
            tc.tile_pool(name="node", bufs=1) as nodep,
            tc.tile_pool(name="qacc", bufs=1, space="PSUM") as qaccp,
            tc.tile_pool(name="rad", bufs=1, space="PSUM") as radp,
            tc.tile_pool(name="ps_epi", bufs=1, space="PSUM") as pse,
        ):
            rep_ctx = tc.For_i(0, reps, 1) if reps > 1 else None
            if rep_ctx is not None:
                rep_ctx.__enter__()
            sT_all = nodep.tile([48, 1024], FP, tag="sT_all")
            gT_all = nodep.tile([48, 1024], FP, tag="gT_all")
            ns_all = nodep.tile([48, 1024], FP, tag="ns_all")
            fino_bufs = []
            for b in range(BUCKETS):
                dlb = bktp.tile([128, T], FP, tag="dl")
                pdb = bktp.tile([128, T * 4], FP, tag="pd")
                xgb = bktp.tile([128, T * 128], BF, tag="xgb")
                r0 = 128 * b
                nc.sync.dma_start(out=dlb[:], in_=d_dl[r0:r0 + 128, :])
                nc.sync.dma_start(out=pdb[:], in_=d_pd[r0:r0 + 128, :])
                if GATHER_MODE == "gather":
                    idxt = bktp.tile([128, T * 8], I16, tag="idx")
                    nc.sync.dma_start(out=idxt[:], in_=d_idx[r0:r0 + 128, :])
                    nc.gpsimd.dma_gather(
                        out_ap=xgb[:].rearrange("p (t e) -> p t e", e=128),
                        in_ap=d_xb[:, :],
                        idxs_ap=idxt[:],
                        num_idxs=T * 128,
                        num_idxs_reg=T * 128,
                        elem_size=128,
                        single_packet=False,
                    )
                else:
                    sidxt = bktp.tile([128, T], I32, tag="sidx")
                    nc.sync.dma_start(out=sidxt[:], in_=d_srcidx[r0:r0 + 128, :])
                    for tt in range(T):
                        nc.gpsimd.indirect_dma_start(
                            out=xgb[:, 128 * tt:128 * (tt + 1)],
                            out_offset=None,
                            in_=d_xb[:],
                            in_offset=bass.IndirectOffsetOnAxis(
                                ap=sidxt[:, tt:tt + 1], axis=0
                            ),
                        )
                # ---- batched edge geometry (whole bucket at once) ----
                vec_all = geop.tile([128, T * 3], FP, tag="vec")
                sq_all = geop.tile([128, T * 3], FP, tag="sq")
                lensq = geop.tile([128, T], FP, tag="lensq")
                len_all = geop.tile([128, T], FP, tag="len")
                invl = geop.tile([128, T], FP, tag="invl")
                y1_all = geop.tile([128, T * 3], FP, tag="y1")
                xgb_t3 = xgb[:].rearrange("p (t e) -> p t e", e=128)
                psrc_f32 = xgb_t3[:, :, 96:102].bitcast(FP)
                nc.vector.tensor_tensor(
                    out=vec_all[:].rearrange("p (t m) -> p t m", m=3),
                    in0=pdb[:].rearrange("p (t m) -> p t m", m=4)[:, :, 0:3],
                    in1=psrc_f32,
                    op=OP.subtract,
                )
                nc.vector.tensor_tensor(
                    out=sq_all[:].rearrange("p (t m) -> p t m", m=3),
                    in0=vec_all[:].rearrange("p (t m) -> p t m", m=3),
                    in1=vec_all[:].rearrange("p (t m) -> p t m", m=3),
                    op=OP.mult,
                )
                nc.vector.reduce_sum(
                    lensq[:], sq_all[:].rearrange("p (t m) -> p t m", m=3),
                    axis=mybir.AxisListType.X,
                )
                nc.scalar.activation(len_all[:], lensq[:], AF.Sqrt)
                nc.vector.tensor_scalar_max(len_all[:], len_all[:], 1e-8)
                nc.vector.reciprocal(invl[:], len_all[:])
                nc.vector.tensor_tensor(
                    out=y1_all[:].rearrange("p (t m) -> p t m", m=3),
                    in0=vec_all[:].rearrange("p (t m) -> p t m", m=3),
                    in1=invl[:].rearrange("p (t m) -> p t m", m=1).to_broadcast(
                        [128, T, 3]
                    ),
                    op=OP.mult,
                )
                # ---- radial coefficients (bucket-batched relu spline) ----
                rad_ps = radp.tile([128, 1536], FP, tag="rad")
                gpre = rad_ps[0:NKNOT, 0:T * 128]
                c_ps = rad_ps[:, T * 128:T * 128 + R * T]
                for t in range(T):
                    # gpre[k, e] = len[e]: transpose-trick matmul with a
                    # broadcast (stride-0) stationary operand
                    nc.tensor.matmul(
                        gpre[:, 128 * t:128 * (t + 1)],
                        lhsT=len_all[:, t:t + 1].to_broadcast([128, NKNOT]),
                        rhs=ident_sb[:],
                        start=True, stop=True,
                    )
                g_sb = geop.tile([NKNOT, T * 128], FP, tag="g_sb")
                nc.scalar.activation(
                    g_sb[:], gpre, AF.Relu, bias=knots_sb[:, 0:1]
                )
                for t in range(T):
                    nc.tensor.matmul(
                        c_ps[:, R * t:R * (t + 1)],
                        lhsT=g_sb[:, 128 * t:128 * (t + 1)],
                        rhs=v_sb[:],
                        start=True, stop=True,
                    )
                c_sb = geop.tile([128, R * T], FP, tag="c_sb")
                nc.scalar.activation(c_sb[:], c_ps, AF.Copy)
                # ---- batched features F_all = [xs | xv | xvy | xsY] ----
                f_all = fallp.tile([128, T * NF], BF, tag="F")
                f_t = f_all[:].rearrange("p (t f) -> p t f", f=NF)
                pvy = fallp.tile([128, T * 48], FP, tag="pvy")
                nc.vector.tensor_copy(f_t[:, :, 0:96], xgb_t3[:, :, 0:96])
                nc.vector.tensor_tensor(
                    out=pvy[:].rearrange("p (t i m) -> p t i m", i=16, m=3),
                    in0=xgb_t3[:, :, 48:96].rearrange(
                        "p t (i m) -> p t i m", m=3
                    ),
                    in1=y1_all[:].rearrange("p (t o m) -> p t o m", o=1, m=3)
                    .to_broadcast([128, T, 16, 3]),
                    op=OP.mult,
                )
                with nc.allow_low_precision(reason="3-term dot, bf16 out"):
                    nc.vector.reduce_sum(
                        f_t[:, :, 96:112],
                        pvy[:].rearrange("p (t i m) -> p t i m", i=16, m=3),
                        axis=mybir.AxisListType.X,
                    )
                nc.vector.tensor_tensor(
                    out=f_t[:, :, 112:256].rearrange(
                        "p t (m i) -> p t m i", i=48
                    ),
                    in0=xgb_t3[:, :, 0:48].rearrange(
                        "p t (o i) -> p t o i", o=1
                    ).to_broadcast([128, T, 3, 48]),
                    in1=y1_all[:].rearrange("p (t m o) -> p t m o", m=3, o=1)
                    .to_broadcast([128, T, 3, 48]),
                    op=OP.mult,
                )
                # ---- per-tile: scaled one-hots + fused TP/scatter ----
                q_ps = qaccp.tile([128, 2048], FP, tag="q")
                for t in range(T):
                    oh_all = ohp.tile([128, OHW], BF, tag="oh")
                    for r in range(R):
                        nc.vector.tensor_scalar(
                            out=oh_all[:, 128 * r:128 * (r + 1)],
                            in0=iota_sb[:],
                            scalar1=dlb[:, t:t + 1],
                            scalar2=c_sb[:, R * t + r:R * t + r + 1],
                            op0=OP.is_equal,
                            op1=OP.mult,
                        )
                    for h in (0, 1):
                        for (c0, c1) in RHS_PARTS:
                            nc.tensor.matmul(
                                q_ps[:, 1024 * h + c0:1024 * h + c1],
                                lhsT=f_all[:, NF * t + 128 * h:
                                           NF * t + 128 * (h + 1)],
                                rhs=oh_all[:, c0:c1],
                                start=(t == 0),
                                stop=(t == T - 1),
                            )
                # ---- bucket epilogue: weight contraction + node linears ----
                qsb = epip.tile([128, NCHUNK * 128], FP, tag="qsb")
                nc.scalar.activation(qsb[:, 0:OHW], q_ps[:, 0:OHW], AF.Copy)
                nc.scalar.activation(
                    qsb[:, OHW:2 * OHW], q_ps[:, 1024:1024 + OHW], AF.Copy
                )
                big_ps = pse.tile([128, 512], FP, tag="big")
                outT_ps = big_ps[:, 0:128]
                sT_ps = big_ps[0:48, 128:256]
                gT_ps = big_ps[0:48, 256:384]
                nsT_ps = big_ps[0:48, 384:512]
                for k in range(NCHUNK):
                    r, h = k // 2, k % 2
                    nc.tensor.matmul(
                        outT_ps,
                        lhsT=wbig_sb[:, 128 * k:128 * (k + 1)],
                        rhs=qsb[:, OHW * h + 128 * r:OHW * h + 128 * (r + 1)],
                        start=(k == 0),
                        stop=(k == NCHUNK - 1),
                    )
                acc_s = epip.tile([48, 128], FP, tag="acc_s")
                acc_v = epip.tile([48, 128], FP, tag="acc_v")
                nc.scalar.activation(acc_s[:], outT_ps[0:48, :], AF.Copy)
                nc.scalar.activation(acc_v[:], outT_ps[64:112, :], AF.Copy)
                nc.tensor.matmul(
                    sT_ps, lhsT=ws_sb[:], rhs=acc_s[:], start=True, stop=True
                )
                nc.tensor.matmul(
                    gT_ps, lhsT=wg_sb[:], rhs=acc_s[:], start=True, stop=True
                )
                nc.tensor.matmul(
                    nsT_ps, lhsT=wns_sb[:], rhs=acc_v[:], start=True, stop=True
                )
                nc.scalar.activation(
                    sT_all[:, 128 * b:128 * (b + 1)], sT_ps, AF.Copy
                )
                nc.scalar.activation(
                    gT_all[:, 128 * b:128 * (b + 1)], gT_ps, AF.Copy
                )
                nc.scalar.activation(
                    ns_all[:, 128 * b:128 * (b + 1)], nsT_ps, AF.Copy
                )
            # ---- batched gated node nonlinearity (one ACT table switch/rep) --
            sig_s = nodep.tile([48, 1024], FP, tag="sig_s")
            sig_g = nodep.tile([48, 1024], FP, tag="sig_g")
            fin_all = nodep.tile([48, 1024], FP, tag="fin_all")
            nc.scalar.activation(sig_s[:], sT_all[:], AF.Sigmoid)
            nc.scalar.activation(sig_g[:], gT_all[:], AF.Sigmoid)
            nc.vector.tensor_tensor(
                out=sig_s[:], in0=sT_all[:], in1=sig_s[:], op=OP.mult
            )
            nc.vector.tensor_tensor(
                out=fin_all[:], in0=sig_g[:], in1=ns_all[:], op=OP.mult
            )
            nc.vector.tensor_tensor(
                out=fin_all[:], in0=fin_all[:], in1=sig_s[:], op=OP.add
            )
            for b in range(BUCKETS):
                big_ps = pse.tile([128, 512], FP, tag="big")
                finT_ps = big_ps[:, 48 * (b % 2):48 * (b % 2) + 48]
                nc.tensor.transpose(
                    finT_ps, fin_all[:, 128 * b:128 * (b + 1)],
                    ident_sb[:48, :48],
                )
                fino = epip.tile([128, 48], FP, tag="fino")
                nc.scalar.activation(fino[:], finT_ps, AF.Copy)
                nc.sync.dma_start(
                    out=d_out[128 * b:128 * (b + 1), :], in_=fino[:]
                )
            if rep_ctx is not None:
                rep_ctx.__exit__(None, None, None)
    nc.finalize()
    return nc


def _make_in_maps(inputs, idx_pair, dl, pdst):
    idx16, srcidx = idx_pair
    import ml_dtypes

    x = np.asarray(inputs["x"], np.float32)
    pos = np.asarray(inputs["pos"], np.float32)
    w1 = np.asarray(inputs["w1"], np.float32)
    w2 = np.asarray(inputs["w2"], np.float32)

    # node table: [x bf16 (96) | pos as raw fp32 bytes (6 bf16 slots) | pad]
    xb = np.zeros((N_NODES, 128), ml_dtypes.bfloat16)
    xb[:, 0:96] = x.astype(ml_dtypes.bfloat16)
    xb.view(np.uint16)[:, 96:102] = (
        pos.astype(np.float32).view(np.uint16).reshape(N_NODES, 6)
    )

    # len_max from actual edge geometry (host gather, cheap)
    ei = np.asarray(inputs["edge_index"], np.int64)
    vec = pos[ei[1]] - pos[ei[0]]
    len_max = float(np.sqrt((vec * vec).sum(axis=1)).max())

    knots, a_relu, br = _radial_basis(w1, w2, len_max)
    wbig = _build_wbig(br)                          # [R*256, 128]
    wbig_packed = np.ascontiguousarray(
        wbig.reshape(NCHUNK, 128, 128).transpose(1, 0, 2).reshape(128, NCHUNK * 128)
    ).astype(np.float32)

    ws_c = (np.asarray(inputs["Ws"], np.float32) / np.sqrt(M0)).astype(np.float32)
    wg_c = (np.asarray(inputs["Wg"], np.float32) / np.sqrt(M0)).astype(np.float32)
    wns_c = _wns_block(np.asarray(inputs["Wns"], np.float32))
    ident = np.eye(128, dtype=np.float32)
    iota = np.tile(np.arange(128, dtype=np.float32), (128, 1)).astype(
        ml_dtypes.bfloat16
    )
    in_maps = []
    for c in range(N_CORES):
        in_maps.append({
            "xb": xb,
            "idx16": np.ascontiguousarray(idx16[c]),
            "srcidx": np.ascontiguousarray(srcidx[c]),
            "dl": np.ascontiguousarray(dl[c]),
            "pdst": np.ascontiguousarray(pdst[c]),
            "wbig": wbig_packed,
            "vmat": a_relu.astype(np.float32),
            "nknots": (-knots.reshape(NKNOT, 1)).astype(np.float32),
            "ws": ws_c, "wg": wg_c, "wns": wns_c,
            "ident": ident, "iota": iota,
        })
    return in_maps


def kernel(x, pos, edge_index, w1, w2, Ws, Wns, Wg):
    inputs = {"x": x, "pos": pos, "edge_index": edge_index, "w1": w1,
              "w2": w2, "Ws": Ws, "Wns": Wns, "Wg": Wg}
    idx16, dl, pdst, T = _prep_edges(
        np.asarray(edge_index, np.int64), np.asarray(pos, np.float32)
    )
    in_maps = _make_in_maps(inputs, idx16, dl, pdst)
    nc = build_kernel(T)
    res = run_bass_kernel_spmd(nc, in_maps, core_ids=list(range(N_CORES)))
    return np.concatenate([res.results[c]["out"] for c in range(N_CORES)], axis=0)


# revision 29
# speedup vs baseline: 2.4355x; 2.4355x over previous
"""EquivariantEdgeConv fused Bass kernel for one TRN2 chip (8 NeuronCores).

Strategy (low-rank radial weights + fused tensor-product/scatter on the PE):
  - The per-edge TP weights w(len) = silu(len*w1) @ w2 / sqrt(H) lie on a
    1-D curve in len; a host-side SVD shows rank R=5 reproduces it to
    ~5e-4, so each edge needs only R radial coefficients.  They are
    evaluated on device via a 64-knot ReLU spline c_r = relu(len-k) @ A
    (relu/sqrt/square/copy share one ACT table set -> no per-tile
    activation-table reloads; the silu/MLP is folded into the host fit).
  - Edges are bucketed by destination (host side), 8 buckets of 128 nodes
    per core, padded to a shared static tile count T.  x (bf16) plus raw
    fp32 pos bytes are fetched with per-tile indirect row gathers.
  - Per bucket, geometry (vec/len/y1) and the feature rows
        F = [ xs(48) | xv(48) | xv.y1(16) | xs x y1(144) ]   (bf16)
    are computed BATCHED across all T tiles (a handful of wide DVE ops).
    The radial coefficients come from a transpose-trick matmul with a
    stride-0 broadcast lhsT (gpre[k,e] = len[e]), one wide ACT Relu, and
    per-tile tiny matmuls against the spline matrix A.
  - The TP contraction is FUSED INTO THE SCATTER: per 128-edge tile the
    DVE builds R scaled one-hots oh_r = (iota == dst)*c_r (one fused
    tensor_scalar each, 4x mode), and the PE accumulates
    Q[f, (r,n)] += F_half^T @ [oh_0|...|oh_R-1] with only two stationary
    loads per tile (wide-N matmuls, one PSUM accumulation group per bank,
    no zeroing matmuls needed).  Per bucket a [R*256 -> 128] fp32 weight
    contraction against the host-built W_big yields the scattered
    messages; the gated o3.Linear node stage runs transposed with its
    sigmoids batched once per iteration (one ACT table switch).
  - Outputs concatenate across cores -- no collective needed.

Self-contained: shapes hardcoded for N=8192, E=65536, irreps 48x0e+16x1o,
H=64.
"""

import sys

if "/opt/trn_rl_repo" not in sys.path:
    sys.path.insert(0, "/opt/trn_rl_repo")

import numpy as np

import concourse.bacc as bacc
import concourse.bass as bass
import concourse.mybir as mybir
import concourse.tile as tile
from concourse.bass_utils import run_bass_kernel_spmd

M0, M1, H = 48, 16, 64
N_NODES, N_EDGES, N_CORES = 8192, 65536, 8
NODES_PER_CORE = N_NODES // N_CORES          # 1024
BUCKETS = NODES_PER_CORE // 128              # 8 buckets of 128 nodes per core
R = 5                                        # radial basis rank
GATHER_MODE = __import__('os').environ.get('GATHER_MODE', 'indirect')  # gather|indirect
NF = 256                                     # per-edge feature width
NCHUNK = R * NF // 128                       # 12 P/W chunks of 128 rows
FP = mybir.dt.float32
BF = mybir.dt.bfloat16
I16 = mybir.dt.int16
I32 = mybir.dt.int32

CA = 1.0 / np.sqrt(M0 * 2.0)
CB = 1.0 / np.sqrt(3.0 * M1 * 2.0)
CC = 1.0 / np.sqrt(M0 * 2.0)
CD = 1.0 / np.sqrt(M1 * 2.0)
SQRT3 = float(np.sqrt(3.0))


def _silu64(x):
    return x / (1.0 + np.exp(-x))


NKNOT = 64


def _radial_basis(w1, w2, len_max):
    """Rank-R SVD basis of the radial weight curve w(len), evaluated on
    device through a ReLU spline: c_r(len) = relu(len - knots) @ A.

    (ReLU is used because sqrt/relu/square/copy share one ACT function
    table set on TRN2 -- no per-tile table reloads.)

    Returns knots [NKNOT], A [NKNOT, R], BR [4096, R], all float64.
    """
    grid = np.linspace(0.0, float(len_max) * 1.02 + 1e-6, 2048)
    hs = _silu64(grid[:, None] * w1.astype(np.float64)[0][None, :])   # [S,H]
    ws = hs @ w2.astype(np.float64) / np.sqrt(H)                      # [S,4096]
    _, _, vt = np.linalg.svd(ws, full_matrices=False)
    br = vt[:R].T                                                     # [4096,R]
    c_true = ws @ br                                                  # [S,R]
    knots = np.linspace(0.0, float(len_max) * 1.01, NKNOT)
    g = np.maximum(grid[:, None] - knots[None, :], 0.0)               # [S,NKNOT]
    a = np.linalg.solve(
        g.T @ g + 1e-7 * np.eye(NKNOT), g.T @ c_true
    )                                                                 # [NKNOT,R]
    return knots, a, br


def _build_wbig(br):
    """W_big [(r,f), q] mapping rank-1 features to the 96 message outputs.

    f layout: [xs(48) | xv(i,m)(48) | xvy(16) | xsY(m-major,144)]
    q layout: [ms o<48 | pad(16) | mv 64+3o+m | pad(16)]  (mv at partition
    base 64 so the epilogue ACT copy reads at a legal partition offset)
    Returns [R*256, 128] float64.
    """
    wb = np.zeros((R, NF, 128))
    # a/b/c/d carry the trailing R axis ([i, o, R])
    a = br[:2304].reshape(48, 48, R)
    b = br[2304:3072].reshape(16, 48, R)
    c = br[3072:3840].reshape(48, 16, R)
    d = br[3840:4096].reshape(16, 16, R)
    for r in range(R):
        # path A: f=i (xs), q=o
        wb[r, 0:48, 0:48] = CA * a[:, :, r]
        # path B: f=96+i (xvy), q=o  (sqrt3 from Y1)
        wb[r, 96:112, 0:48] = CB * SQRT3 * b[:, :, r]
        # path D: f=48+3i+m (xv), q=64+3o+m
        for m in range(3):
            wb[r, 48 + m:96:3, 64 + m:112:3] = CD * d[:, :, r]
            # path C: f=112+48m+i (xsY), q=64+3o+m  (sqrt3 from Y1)
            wb[r, 112 + 48 * m:160 + 48 * m, 64 + m:112:3] = CC * SQRT3 * c[:, :, r]
    return wb.reshape(R * NF, 128)


def _wns_block(wns):
    """[48,48] lhsT for the 1o o3.Linear on (o,m)-interleaved rows."""
    out = np.zeros((48, 48), np.float32)
    for i in range(16):
        for m in range(3):
            for o in range(16):
                out[i * 3 + m, o * 3 + m] = wns[i, o] / np.sqrt(M1)
    return out


def _prep_edges(edge_index, pos):
    """Bucket/pad edges by destination.

    Returns per-core arrays:
      idx16  [N_CORES, BUCKETS*128, T*8]  int16  (dma_gather wrapped+replicated)
      dl     [N_CORES, BUCKETS*128, T]    fp32   (local dst, 300 for padding)
      pdst   [N_CORES, BUCKETS*128, T*4]  fp32   (pos[dst], w-padded)
    and the shared tiles-per-bucket count T.
    """
    src = edge_index[0].astype(np.int64)
    dst = edge_index[1].astype(np.int64)
    gb = dst >> 7
    order = np.argsort(gb, kind="stable")
    src_s, dst_s = src[order], dst[order]
    counts = np.bincount(gb[order], minlength=64)
    cap = max(int(np.ceil(counts.max() / 128) * 128), 128)
    T = cap // 128
    starts = np.concatenate([[0], np.cumsum(counts)])

    pos = np.asarray(pos, np.float32)
    idx16 = np.zeros((N_CORES, BUCKETS * 128, T * 8), np.int16)
    srcidx = np.zeros((N_CORES, BUCKETS * 128, T), np.int32)
    dl = np.full((N_CORES, BUCKETS * 128, T), 300.0, np.float32)
    pdst = np.zeros((N_CORES, BUCKETS * 128, T * 4), np.float32)

    for g in range(64):
        ccore, b = g >> 3, g & 7
        s, e = starts[g], starts[g + 1]
        n = e - s
        sidx = np.zeros(cap, np.int64)
        sidx[:n] = src_s[s:e]
        dloc = np.full(cap, 300.0, np.float32)
        dloc[:n] = (dst_s[s:e] - (g << 7)).astype(np.float32)
        pd = np.zeros((cap, 3), np.float32)
        pd[:n] = pos[dst_s[s:e]]
        pd[n:] = pos[0]  # padding: same as pos[src=0] so vec==0, no NaNs
        # edge k -> partition k%128, tile k//128
        k = np.arange(cap)
        p, t = k % 128, k // 128
        r0 = 128 * b
        dl[ccore, r0 + p, t] = dloc
        srcidx[ccore, r0 + p, t] = sidx.astype(np.int32)
        pdst[ccore, r0 + p[:, None], 4 * t[:, None] + np.arange(3)[None, :]] = pd
        # gather idx wrap: idx k -> [k%16, k//16], replicated to 128 partitions
        wrapped = np.zeros((16, T * 8), np.int16)
        wrapped[k % 16, k // 16] = sidx.astype(np.int16)
        idx16[ccore, r0:r0 + 128, :] = np.tile(wrapped, (8, 1))
    return (idx16, srcidx), dl, pdst, T


def build_kernel(tiles_per_bucket: int, reps: int = 1) -> bass.Bass:
    T = tiles_per_bucket
    assert T <= 10, "radial PSUM layout sized for T<=10"
    OHW = R * 128                         # scaled-onehot width (768)
    RHS_PARTS = [(0, min(512, OHW))] + ([(512, OHW)] if OHW > 512 else [])
    nc = bacc.Bacc(None, target_bir_lowering=False, debug=False)
    d_xb = nc.declare_dram_parameter("xb", [N_NODES, 128], BF, isOutput=False)
    d_idx = nc.declare_dram_parameter("idx16", [BUCKETS * 128, T * 8], I16, isOutput=False)
    d_srcidx = nc.declare_dram_parameter("srcidx", [BUCKETS * 128, T], I32, isOutput=False)
    d_dl = nc.declare_dram_parameter("dl", [BUCKETS * 128, T], FP, isOutput=False)
    d_pd = nc.declare_dram_parameter("pdst", [BUCKETS * 128, T * 4], FP, isOutput=False)
    d_wbig = nc.declare_dram_parameter("wbig", [128, NCHUNK * 128], FP, isOutput=False)
    d_v = nc.declare_dram_parameter("vmat", [NKNOT, R], FP, isOutput=False)
    d_knots = nc.declare_dram_parameter("nknots", [NKNOT, 1], FP, isOutput=False)
    d_ws = nc.declare_dram_parameter("ws", [M0, M0], FP, isOutput=False)
    d_wg = nc.declare_dram_parameter("wg", [M0, M0], FP, isOutput=False)
    d_wns = nc.declare_dram_parameter("wns", [48, 48], FP, isOutput=False)
    d_ident = nc.declare_dram_parameter("ident", [128, 128], FP, isOutput=False)
    d_iota = nc.declare_dram_parameter("iota", [128, 128], BF, isOutput=False)
    d_out = nc.declare_dram_parameter("out", [NODES_PER_CORE, M0], FP, isOutput=True)

    AF = mybir.ActivationFunctionType
    OP = mybir.AluOpType

    with tile.TileContext(nc) as tc, tc.tile_pool(name="consts", bufs=1) as cp:
        wbig_sb = cp.tile([128, NCHUNK * 128], FP)
        v_sb = cp.tile([NKNOT, R], FP)
        knots_sb = cp.tile([NKNOT, 1], FP)
        ws_sb = cp.tile([M0, M0], FP)
        wg_sb = cp.tile([M0, M0], FP)
        wns_sb = cp.tile([48, 48], FP)
        ident_sb = cp.tile([128, 128], FP)
        iota_sb = cp.tile([128, 128], BF)
        for sb, dr in (
            (wbig_sb, d_wbig), (v_sb, d_v), (knots_sb, d_knots), (ws_sb, d_ws),
            (wg_sb, d_wg), (wns_sb, d_wns), (ident_sb, d_ident), (iota_sb, d_iota),
        ):
            nc.sync.dma_start(out=sb[:], in_=dr[:])

        with (
            tc.tile_pool(name="bkt", bufs=2) as bktp,
            tc.tile_pool(name="geo", bufs=2) as geop,
            tc.tile_pool(name="fall", bufs=2) as fallp,
            tc.tile_pool(name="ohp", bufs=3) as ohp,
            tc.tile_pool(name="epi", bufs=2) as epip,
            tc.tile_pool(name="node", bufs=1) as nodep,
            tc.tile_pool(name="qacc", bufs=1, space="PSUM") as qaccp,
            tc.tile_pool(name="rad", bufs=1, space="PSUM") as radp,
            tc.tile_pool(name="ps_epi", bufs=1, space="PSUM") as pse,
        ):
            rep_ctx = tc.For_i(0, reps, 1) if reps > 1 else None
            if rep_ctx is not None:
                rep_ctx.__enter__()
            sT_all = nodep.tile([48, 1024], FP, tag="sT_all")
            gT_all = nodep.tile([48, 1024], FP, tag="gT_all")
            ns_all = nodep.tile([48, 1024], FP, tag="ns_all")
            fino_bufs = []
            for b in range(BUCKETS):
                dlb = bktp.tile([128, T], FP, tag="dl")
                pdb = bktp.tile([128, T * 4], FP, tag="pd")
                xgb = bktp.tile([128, T * 128], BF, tag="xgb")
                r0 = 128 * b
                nc.sync.dma_start(out=dlb[:], in_=d_dl[r0:r0 + 128, :])
                nc.sync.dma_start(out=pdb[:], in_=d_pd[r0:r0 + 128, :])
                if GATHER_MODE == "gather":
                    idxt = bktp.tile([128, T * 8], I16, tag="idx")
                    nc.sync.dma_start(out=idxt[:], in_=d_idx[r0:r0 + 128, :])
                    nc.gpsimd.dma_gather(
                        out_ap=xgb[:].rearrange("p (t e) -> p t e", e=128),
                        in_ap=d_xb[:, :],
                        idxs_ap=idxt[:],
                        num_idxs=T * 128,
                        num_idxs_reg=T * 128,
                        elem_size=128,
                        single_packet=False,
                    )
                else:
                    sidxt = bktp.tile([128, T], I32, tag="sidx")
                    nc.sync.dma_start(out=sidxt[:], in_=d_srcidx[r0:r0 + 128, :])
                    for tt in range(T):
                        nc.gpsimd.indirect_dma_start(
                            out=xgb[:, 128 * tt:128 * (tt + 1)],
                            out_offset=None,
                            in_=d_xb[:],
                            in_offset=bass.IndirectOffsetOnAxis(
                                ap=sidxt[:, tt:tt + 1], axis=0
                            ),
                        )
                # ---- batched edge geometry (whole bucket at once) ----
                vec_all = geop.tile([128, T * 3], FP, tag="vec")
                sq_all = geop.tile([128, T * 3], FP, tag="sq")
                lensq = geop.tile([128, T], FP, tag="lensq")
                len_all = geop.tile([128, T], FP, tag="len")
                invl = geop.tile([128, T], FP, tag="invl")
                y1_all = geop.tile([128, T * 3], FP, tag="y1")
                xgb_t3 = xgb[:].rearrange("p (t e) -> p t e", e=128)
                psrc_f32 = xgb_t3[:, :, 96:102].bitcast(FP)
                nc.vector.tensor_tensor(
                    out=vec_all[:].rearrange("p (t m) -> p t m", m=3),
                    in0=pdb[:].rearrange("p (t m) -> p t m", m=4)[:, :, 0:3],
                    in1=psrc_f32,
                    op=OP.subtract,
                )
                nc.vector.tensor_tensor(
                    out=sq_all[:].rearrange("p (t m) -> p t m", m=3),
                    in0=vec_all[:].rearrange("p (t m) -> p t m", m=3),
                    in1=vec_all[:].rearrange("p (t m) -> p t m", m=3),
                    op=OP.mult,
                )
                nc.vector.reduce_sum(
                    lensq[:], sq_all[:].rearrange("p (t m) -> p t m", m=3),
                    axis=mybir.AxisListType.X,
                )
                nc.scalar.activation(len_all[:], lensq[:], AF.Sqrt)
                nc.vector.tensor_scalar_max(len_all[:], len_all[:], 1e-8)
                nc.vector.reciprocal(invl[:], len_all[:])
                nc.vector.tensor_tensor(
                    out=y1_all[:].rearrange("p (t m) -> p t m", m=3),
                    in0=vec_all[:].rearrange("p (t m) -> p t m", m=3),
                    in1=invl[:].rearrange("p (t m) -> p t m", m=1).to_broadcast(
                        [128, T, 3]
                    ),
                    op=OP.mult,
                )
                # ---- radial coefficients (bucket-batched relu spline) ----
                rad_ps = radp.tile([128, 1536], FP, tag="rad")
                gpre = rad_ps[0:NKNOT, 0:T * 128]
                c_ps = rad_ps[:, T * 128:T * 128 + R * T]
                for t in range(T):
                    # gpre[k, e] = len[e]: transpose-trick matmul with a
                    # broadcast (stride-0) stationary operand
                    nc.tensor.matmul(
                        gpre[:, 128 * t:128 * (t + 1)],
                        lhsT=len_all[:, t:t + 1].to_broadcast([128, NKNOT]),
                        rhs=ident_sb[:],
                        start=True, stop=True,
                    )
                g_sb = geop.tile([NKNOT, T * 128], FP, tag="g_sb")
                nc.scalar.activation(
                    g_sb[:], gpre, AF.Relu, bias=knots_sb[:, 0:1]
                )
                for t in range(T):
                    nc.tensor.matmul(
                        c_ps[:, R * t:R * (t + 1)],
                        lhsT=g_sb[:, 128 * t:128 * (t + 1)],
                        rhs=v_sb[:],
                        start=True, stop=True,
                    )
                c_sb = geop.tile([128, R * T], FP, tag="c_sb")
                nc.scalar.activation(c_sb[:], c_ps, AF.Copy)
                # ---- batched features F_all = [xs | xv | xvy | xsY] ----
                f_all = fallp.tile([128, T * NF], BF, tag="F")
                f_t = f_all[:].rearrange("p (t f) -> p t f", f=NF)
                pvy = fallp.tile([128, T * 48], FP, tag="pvy")
                nc.vector.tensor_copy(f_t[:, :, 0:96], xgb_t3[:, :, 0:96])
                nc.vector.tensor_tensor(
                    out=pvy[:].rearrange("p (t i m) -> p t i m", i=16, m=3),
                    in0=xgb_t3[:, :, 48:96].rearrange(
                        "p t (i m) -> p t i m", m=3
                    ),
                    in1=y1_all[:].rearrange("p (t o m) -> p t o m", o=1, m=3)
                    .to_broadcast([128, T, 16, 3]),
                    op=OP.mult,
                )
                with nc.allow_low_precision(reason="3-term dot, bf16 out"):
                    nc.vector.reduce_sum(
                        f_t[:, :, 96:112],
                        pvy[:].rearrange("p (t i m) -> p t i m", i=16, m=3),
                        axis=mybir.AxisListType.X,
                    )
                nc.vector.tensor_tensor(
                    out=f_t[:, :, 112:256].rearrange(
                        "p t (m i) -> p t m i", i=48
                    ),
                    in0=xgb_t3[:, :, 0:48].rearrange(
                        "p t (o i) -> p t o i", o=1
                    ).to_broadcast([128, T, 3, 48]),
                    in1=y1_all[:].rearrange("p (t m o) -> p t m o", m=3, o=1)
                    .to_broadcast([128, T, 3, 48]),
                    op=OP.mult,
                )
                # ---- per-tile: scaled one-hots + fused TP/scatter ----
                q_ps = qaccp.tile([128, 2048], FP, tag="q")
                for t in range(T):
                    oh_all = ohp.tile([128, OHW], BF, tag="oh")
                    for r in range(R):
                        nc.vector.tensor_scalar(
                            out=oh_all[:, 128 * r:128 * (r + 1)],
                            in0=iota_sb[:],
                            scalar1=dlb[:, t:t + 1],
                            scalar2=c_sb[:, R * t + r:R * t + r + 1],
                            op0=OP.is_equal,
                            op1=OP.mult,
                        )
                    for h in (0, 1):
                        for (c0, c1) in RHS_PARTS:
                            nc.tensor.matmul(
                                q_ps[:, 1024 * h + c0:1024 * h + c1],
                                lhsT=f_all[:, NF * t + 128 * h:
                                           NF * t + 128 * (h + 1)],
                                rhs=oh_all[:, c0:c1],
                                start=(t == 0),
                                stop=(t == T - 1),
                            )
                # ---- bucket epilogue: weight contraction + node linears ----
                qsb = epip.tile([128, NCHUNK * 128], FP, tag="qsb")
                nc.scalar.activation(qsb[:, 0:OHW], q_ps[:, 0:OHW], AF.Copy)
                nc.scalar.activation(
                    qsb[:, OHW:2 * OHW], q_ps[:, 1024:1024 + OHW], AF.Copy
                )
                big_ps = pse.tile([128, 512], FP, tag="big")
                outT_ps = big_ps[:, 0:128]
                sT_ps = big_ps[0:48, 128:256]
                gT_ps = big_ps[0:48, 256:384]
                nsT_ps = big_ps[0:48, 384:512]
                for k in range(NCHUNK):
                    r, h = k // 2, k % 2
                    nc.tensor.matmul(
                        outT_ps,
                        lhsT=wbig_sb[:, 128 * k:128 * (k + 1)],
                        rhs=qsb[:, OHW * h + 128 * r:OHW * h + 128 * (r + 1)],
                        start=(k == 0),
                        stop=(k == NCHUNK - 1),
                    )
                acc_s = epip.tile([48, 128], FP, tag="acc_s")
                acc_v = epip.tile([48, 128], FP, tag="acc_v")
                nc.scalar.activation(acc_s[:], outT_ps[0:48, :], AF.Copy)
                nc.scalar.activation(acc_v[:], outT_ps[64:112, :], AF.Copy)
                nc.tensor.matmul(
                    sT_ps, lhsT=ws_sb[:], rhs=acc_s[:], start=True, stop=True
                )
                nc.tensor.matmul(
                    gT_ps, lhsT=wg_sb[:], rhs=acc_s[:], start=True, stop=True
                )
                nc.tensor.matmul(
                    nsT_ps, lhsT=wns_sb[:], rhs=acc_v[:], start=True, stop=True
                )
                nc.scalar.activation(
                    sT_all[:, 128 * b:128 * (b + 1)], sT_ps, AF.Copy
                )
                nc.scalar.activation(
                    gT_all[:, 128 * b:128 * (b + 1)], gT_ps, AF.Copy
                )
                nc.scalar.activation(
                    ns_all[:, 128 * b:128 * (b + 1)], nsT_ps, AF.Copy
                )
            # ---- batched gated node nonlinearity (one ACT table switch/rep) --
            sig_s = nodep.tile([48, 1024], FP, tag="sig_s")
            sig_g = nodep.tile([48, 1024], FP, tag="sig_g")
            fin_all = nodep.tile([48, 1024], FP, tag="fin_all")
            nc.scalar.activation(sig_s[:], sT_all[:], AF.Sigmoid)
            nc.scalar.activation(sig_g[:], gT_all[:], AF.Sigmoid)
            nc.vector.tensor_tensor(
                out=sig_s[:], in0=sT_all[:], in1=sig_s[:], op=OP.mult
            )
            nc.vector.tensor_tensor(
                out=fin_all[:], in0=sig_g[:], in1=ns_all[:], op=OP.mult
            )
            nc.vector.tensor_tensor(
                out=fin_all[:], in0=fin_all[:], in1=sig_s[:], op=OP.add
            )
            for b in range(BUCKETS):
                big_ps = pse.tile([128, 512], FP, tag="big")
                finT_ps = big_ps[:, 48 * (b % 2):48 * (b % 2) + 48]
                nc.tensor.transpose(
                    finT_ps, fin_all[:, 128 * b:128 * (b + 1)],
                    ident_sb[:48, :48],
                )
                fino = epip.tile([128, 48], FP, tag="fino")
                nc.scalar.activation(fino[:], finT_ps, AF.Copy)
                nc.sync.dma_start(
                    out=d_out[128 * b:128 * (b + 1), :], in_=fino[:]
                )
            if rep_ctx is not None:
                rep_ctx.__exit__(None, None, None)
    nc.finalize()
    return nc


def _make_in_maps(inputs, idx_pair, dl, pdst):
    idx16, srcidx = idx_pair
    import ml_dtypes

    x = np.asarray(inputs["x"], np.float32)
    pos = np.asarray(inputs["pos"], np.float32)
    w1 = np.asarray(inputs["w1"], np.float32)
    w2 = np.asarray(inputs["w2"], np.float32)

    # node table: [x bf16 (96) | pos as raw fp32 bytes (6 bf16 slots) | pad]
    xb = np.zeros((N_NODES, 128), ml_dtypes.bfloat16)
    xb[:, 0:96] = x.astype(ml_dtypes.bfloat16)
    xb.view(np.uint16)[:, 96:102] = (
        pos.astype(np.float32).view(np.uint16).reshape(N_NODES, 6)
    )

    # len_max from actual edge geometry (host gather, cheap)
    ei = np.asarray(inputs["edge_index"], np.int64)
    vec = pos[ei[1]] - pos[ei[0]]
    len_max = float(np.sqrt((vec * vec).sum(axis=1)).max())

    knots, a_relu, br = _radial_basis(w1, w2, len_max)
    wbig = _build_wbig(br)                          # [R*256, 128]
    wbig_packed = np.ascontiguousarray(
        wbig.reshape(NCHUNK, 128, 128).transpose(1, 0, 2).reshape(128, NCHUNK * 128)
    ).astype(np.float32)

    ws_c = (np.asarray(inputs["Ws"], np.float32) / np.sqrt(M0)).astype(np.float32)
    wg_c = (np.asarray(inputs["Wg"], np.float32) / np.sqrt(M0)).astype(np.float32)
    wns_c = _wns_block(np.asarray(inputs["Wns"], np.float32))
    ident = np.eye(128, dtype=np.float32)
    iota = np.tile(np.arange(128, dtype=np.float32), (128, 1)).astype(
        ml_dtypes.bfloat16
    )
    in_maps = []
    for c in range(N_CORES):
        in_maps.append({
            "xb": xb,
            "idx16": np.ascontiguousarray(idx16[c]),
            "srcidx": np.ascontiguousarray(srcidx[c]),
            "dl": np.ascontiguousarray(dl[c]),
            "pdst": np.ascontiguousarray(pdst[c]),
            "wbig": wbig_packed,
            "vmat": a_relu.astype(np.float32),
            "nknots": (-knots.reshape(NKNOT, 1)).astype(np.float32),
            "ws": ws_c, "wg": wg_c, "wns": wns_c,
            "ident": ident, "iota": iota,
        })
    return in_maps


def kernel(x, pos, edge_index, w1, w2, Ws, Wns, Wg):
    inputs = {"x": x, "pos": pos, "edge_index": edge_index, "w1": w1,
              "w2": w2, "Ws": Ws, "Wns": Wns, "Wg": Wg}
    idx16, dl, pdst, T = _prep_edges(
        np.asarray(edge_index, np.int64), np.asarray(pos, np.float32)
    )
    in_maps = _make_in_maps(inputs, idx16, dl, pdst)
    nc = build_kernel(T)
    res = run_bass_kernel_spmd(nc, in_maps, core_ids=list(range(N_CORES)))
    return np.concatenate([res.results[c]["out"] for c in range(N_CORES)], axis=0)
